# revision 9
# baseline (speedup 1.0000x reference)
"""Top-k threshold masking kernel for Trainium2 (Bass/Tile).

Computes, per row of x [2048, 32768] f32:
    threshold = (k+1)-th largest value of the row
    out = where(x >= threshold, x * 10, x)

Sharding: pure data-parallel over the batch (row) dim across 8 NeuronCores
(256 rows/core). Each core:
  - streams its [256, 32768] shard through SBUF in [128, 32768] row-tiles,
  - per free-dim chunk, computes the chunk top-8 with the DVE max8 op while
    the DMA load streams in,
  - merges chunk top-8s -> row top-8 -> threshold = top8[:, k],
  - second pass over resident SBUF data: mask = (x >= thr) on DVE,
    x10 = 10*x on the scalar engine, patch x in place via copy_predicated,
    and DMA the patched tile out.
Data is read from HBM exactly once and written exactly once (memory-bound
regime; ~64 MiB of HBM traffic per core).
"""

import numpy as np

import concourse.bacc as bacc
import concourse.bass as bass
import concourse.mybir as mybir
from concourse.bass_utils import run_bass_kernel_spmd
from concourse.tile import TileContext

N_CORES = 8
B, N = 2048, 32768
ROWS_PER_CORE = B // N_CORES  # 256
P = 128
TILES_PER_CORE = ROWS_PER_CORE // P  # 2
CHUNK = 4096
N_CHUNKS = N // CHUNK

_nc_cache: dict[int, bass.Bass] = {}


def _build(k: int) -> bass.Bass:
    assert 0 <= k <= 7, f"k={k} needs top-(k+1) which must fit in max8's top-8"
    nc = bacc.Bacc("TRN2", target_bir_lowering=False)
    x = nc.dram_tensor("x", [ROWS_PER_CORE, N], mybir.dt.float32, kind="ExternalInput")
    out = nc.dram_tensor(
        "out", [ROWS_PER_CORE, N], mybir.dt.float32, kind="ExternalOutput"
    )

    with TileContext(nc) as tc:
        with (
            tc.tile_pool(name="big", bufs=1) as big_pool,
            tc.tile_pool(name="chunks", bufs=2) as chunk_pool,
            tc.tile_pool(name="small", bufs=2) as small_pool,
        ):
            for t in range(TILES_PER_CORE):
                rows = slice(t * P, (t + 1) * P)
                xt = big_pool.tile([P, N], mybir.dt.float32, tag="xt")
                cand = small_pool.tile([P, 8 * N_CHUNKS], mybir.dt.float32, tag="cand")
                for c in range(N_CHUNKS):
                    sl = slice(c * CHUNK, (c + 1) * CHUNK)
                    nc.sync.dma_start(out=xt[:, sl], in_=x[rows, sl])
                    nc.vector.max(out=cand[:, c * 8 : (c + 1) * 8], in_=xt[:, sl])
                top8 = small_pool.tile([P, 8], mybir.dt.float32, tag="top8")
                nc.vector.max(out=top8, in_=cand)
                thr = top8[:, k : k + 1]
                for c in range(N_CHUNKS):
                    sl = slice(c * CHUNK, (c + 1) * CHUNK)
                    x10 = chunk_pool.tile([P, CHUNK], mybir.dt.float32, tag="x10")
                    mask = chunk_pool.tile([P, CHUNK], mybir.dt.uint8, tag="mask")
                    nc.scalar.mul(x10, xt[:, sl], 10.0)
                    nc.gpsimd.tensor_scalar(
                        mask, xt[:, sl], thr, None, op0=mybir.AluOpType.is_lt
                    )
                    nc.vector.copy_predicated(x10, mask, xt[:, sl])
                    nc.sync.dma_start(out=out[rows, sl], in_=x10)
    nc.compile()
    return nc


def kernel(x: np.ndarray, k) -> np.ndarray:
    k = int(k)
    if k not in _nc_cache:
        _nc_cache[k] = _build(k)
    nc = _nc_cache[k]

    x = np.ascontiguousarray(x, dtype=np.float32)
    in_maps = [
        {"x": x[i * ROWS_PER_CORE : (i + 1) * ROWS_PER_CORE]} for i in range(N_CORES)
    ]
    res = run_bass_kernel_spmd(nc, in_maps, core_ids=list(range(N_CORES)))
    return np.concatenate([r["out"] for r in res.results], axis=0)


# revision 10
# speedup vs baseline: 4.4509x; 4.4509x over previous
"""Top-k threshold masking kernel for Trainium2 (Bass/Tile).

Computes, per row of x [2048, 32768] f32:
    threshold t = (k+1)-th largest value of the row   (k=3 -> 4th largest)
    out = where(x >= t, x * 10, x)

Sharding: pure data-parallel over rows across 8 NeuronCores (256 rows/core).

Per-core plan (memory-bound target; HBM traffic = read once + write once):
  - rows are processed in [128, 32768] row-tiles, streamed through SBUF as
    four [128, 8192] subtiles from a 5-slot pool (the 5th slot lets the next
    row-tile's loads overlap the current tile's phase 2),
  - phase 1: per subtile, DVE max8 computes the subtile top-8 while loads
    stream in; a tiny max8 over the 4x8 candidates gives the row top-8 and
    t = top8[:, k],
  - phase 2 per [128, 2048] chunk:
      ACT: mask_lt = relu((t - x) * 2^24) cast to uint32
           (exact: 2^24 is a power of two so x*2^24 and t*2^24 are exact fp32;
            the sum is correctly rounded so sign/zeroness is exact; the
            smallest nonzero (t-x) is ulp(t) >= 2^-24 for t >= 0.5, so the
            scaled value is >= 1 and the uint32 cast keeps it nonzero.
            Requires 0.5 <= t < 64 - comfortably true for the 4th largest of
            32768 N(0,1) samples),
      ACT: x10 = 10 * x
      DVE: copy_predicated(x10, mask_lt, x)  -> x10 holds the final output
      DMA out x10.
  Engine busy/core est: DMA ~194us (binds), DVE ~140us, ACT ~120us.
"""

import numpy as np

import concourse.bacc as bacc
import concourse.bass as bass
import concourse.mybir as mybir
from concourse.bass_utils import run_bass_kernel_spmd
from concourse.tile import TileContext

N_CORES = 8
B, N = 2048, 32768
ROWS_PER_CORE = B // N_CORES  # 256
P = 128
TILES_PER_CORE = ROWS_PER_CORE // P  # 2
SUB = 8192  # subtile free-dim (DMA/residency unit)
N_SUB = N // SUB  # 4
CHUNK = 2048  # phase-2 compute chunk
CHUNKS_PER_SUB = SUB // CHUNK  # 4
SCALE = float(2**24)

_nc_cache: dict[int, bass.Bass] = {}


def _build(k: int) -> bass.Bass:
    assert 0 <= k <= 7, f"k={k} needs top-(k+1) which must fit in max8's top-8"
    nc = bacc.Bacc("TRN2", target_bir_lowering=False)
    x = nc.dram_tensor("x", [ROWS_PER_CORE, N], mybir.dt.float32, kind="ExternalInput")
    out = nc.dram_tensor(
        "out", [ROWS_PER_CORE, N], mybir.dt.float32, kind="ExternalOutput"
    )

    with TileContext(nc) as tc:
        with (
            tc.tile_pool(name="sub", bufs=5) as sub_pool,
            tc.tile_pool(name="chunks", bufs=2) as chunk_pool,
            tc.tile_pool(name="small", bufs=2) as small_pool,
        ):
            for t in range(TILES_PER_CORE):
                rows = slice(t * P, (t + 1) * P)
                subs = []
                cand = small_pool.tile([P, 8 * N_SUB], mybir.dt.float32, tag="cand")
                for s in range(N_SUB):
                    ssl = slice(s * SUB, (s + 1) * SUB)
                    xs = sub_pool.tile([P, SUB], mybir.dt.float32, tag="xt")
                    nc.sync.dma_start(out=xs, in_=x[rows, ssl])
                    nc.vector.max(out=cand[:, s * 8 : (s + 1) * 8], in_=xs)
                    subs.append(xs)
                top8 = small_pool.tile([P, 8], mybir.dt.float32, tag="top8")
                nc.vector.max(out=top8, in_=cand)
                # bias for the ACT mask: t * 2^24 (exact, power-of-two scale)
                thr_b = small_pool.tile([P, 1], mybir.dt.float32, tag="thr_b")
                nc.vector.tensor_scalar_mul(thr_b, top8[:, k : k + 1], SCALE)
                for s in range(N_SUB):
                    for c in range(CHUNKS_PER_SUB):
                        csl = slice(c * CHUNK, (c + 1) * CHUNK)
                        osl = slice(s * SUB + c * CHUNK, s * SUB + (c + 1) * CHUNK)
                        x10 = chunk_pool.tile([P, CHUNK], mybir.dt.float32, tag="x10")
                        mask = chunk_pool.tile([P, CHUNK], mybir.dt.uint32, tag="mask")
                        # mask_lt = relu(-2^24 * x + 2^24 * t): nonzero iff x < t
                        nc.scalar.activation(
                            mask,
                            subs[s][:, csl],
                            mybir.ActivationFunctionType.Relu,
                            bias=thr_b[:, 0:1],
                            scale=-SCALE,
                        )
                        nc.scalar.mul(x10, subs[s][:, csl], 10.0)
                        nc.vector.copy_predicated(x10, mask, subs[s][:, csl])
                        nc.sync.dma_start(out=out[rows, osl], in_=x10)
    nc.compile()
    return nc


def kernel(x: np.ndarray, k) -> np.ndarray:
    k = int(k)
    if k not in _nc_cache:
        _nc_cache[k] = _build(k)
    nc = _nc_cache[k]

    x = np.ascontiguousarray(x, dtype=np.float32)
    in_maps = [
        {"x": x[i * ROWS_PER_CORE : (i + 1) * ROWS_PER_CORE]} for i in range(N_CORES)
    ]
    res = run_bass_kernel_spmd(nc, in_maps, core_ids=list(range(N_CORES)))
    return np.concatenate([r["out"] for r in res.results], axis=0)


# revision 11
# speedup vs baseline: 5.1594x; 1.1592x over previous
"""Top-k threshold masking kernel for Trainium2 (Bass/Tile).

Computes, per row of x [2048, 32768] f32:
    threshold t = (k+1)-th largest value of the row   (k=3 -> 4th largest)
    out = where(x >= t, x * 10, x)

Sharding: pure data-parallel over rows across 8 NeuronCores (256 rows/core).

Per-core plan (memory-bound target; HBM traffic = read once + write once):
  - rows are processed in [128, 32768] row-tiles, streamed through SBUF as
    four [128, 8192] subtiles from a 5-slot pool (the 5th slot lets the next
    row-tile's loads overlap the current tile's phase 2),
  - phase 1: per subtile, DVE max8 computes the subtile top-8 while loads
    stream in; a max8 over the 4x8 candidates gives the row top-8 and
    t = top8[:, k],
  - phase 2 per [128, 4096] chunk:
      ACT: x10 = 10 * x
      DVE: mask_ge = (x >= t) as uint8 {0,1}   (tensor_scalar, 2x mode)
      DVE: copy_predicated(subtile_chunk, mask_ge, x10)  - in-place patch
    then one 4 MiB out-DMA per subtile. In-place patching keeps the x10/mask
    slot recycle off the DMA critical path (slots free on DVE completion).
  Engine busy/core est: DVE ~175us, DMA ~170us, ACT ~60us.
"""

import numpy as np

import concourse.bacc as bacc
import concourse.bass as bass
import concourse.mybir as mybir
from concourse.bass_utils import run_bass_kernel_spmd
from concourse.tile import TileContext

N_CORES = 8
B, N = 2048, 32768
ROWS_PER_CORE = B // N_CORES  # 256
P = 128
TILES_PER_CORE = ROWS_PER_CORE // P  # 2
SUB = 8192  # subtile free-dim (DMA/residency unit)
N_SUB = N // SUB  # 4
CHUNK = 4096  # phase-2 compute chunk
CHUNKS_PER_SUB = SUB // CHUNK  # 2

_nc_cache: dict[int, bass.Bass] = {}


def _build(k: int) -> bass.Bass:
    assert 0 <= k <= 7, f"k={k} needs top-(k+1) which must fit in max8's top-8"
    nc = bacc.Bacc("TRN2", target_bir_lowering=False)
    x = nc.dram_tensor("x", [ROWS_PER_CORE, N], mybir.dt.float32, kind="ExternalInput")
    out = nc.dram_tensor(
        "out", [ROWS_PER_CORE, N], mybir.dt.float32, kind="ExternalOutput"
    )

    with TileContext(nc) as tc:
        with (
            tc.tile_pool(name="sub", bufs=5) as sub_pool,
            tc.tile_pool(name="chunks", bufs=2) as chunk_pool,
            tc.tile_pool(name="small", bufs=2) as small_pool,
        ):
            for t in range(TILES_PER_CORE):
                rows = slice(t * P, (t + 1) * P)
                subs = []
                cand = small_pool.tile([P, 8 * N_SUB], mybir.dt.float32, tag="cand")
                for s in range(N_SUB):
                    ssl = slice(s * SUB, (s + 1) * SUB)
                    xs = sub_pool.tile([P, SUB], mybir.dt.float32, tag="xt")
                    nc.sync.dma_start(out=xs, in_=x[rows, ssl])
                    nc.vector.max(out=cand[:, s * 8 : (s + 1) * 8], in_=xs)
                    subs.append(xs)
                top8 = small_pool.tile([P, 8], mybir.dt.float32, tag="top8")
                nc.vector.max(out=top8, in_=cand)
                thr = top8[:, k : k + 1]
                for s in range(N_SUB):
                    for c in range(CHUNKS_PER_SUB):
                        csl = slice(c * CHUNK, (c + 1) * CHUNK)
                        x10 = chunk_pool.tile([P, CHUNK], mybir.dt.float32, tag="x10")
                        mask = chunk_pool.tile([P, CHUNK], mybir.dt.uint8, tag="mask")
                        nc.scalar.mul(x10, subs[s][:, csl], 10.0)
                        nc.vector.tensor_scalar(
                            mask, subs[s][:, csl], thr, None, op0=mybir.AluOpType.is_ge
                        )
                        nc.vector.copy_predicated(subs[s][:, csl], mask, x10)
                    nc.sync.dma_start(
                        out=out[rows, s * SUB : (s + 1) * SUB], in_=subs[s]
                    )
    nc.compile()
    return nc


def kernel(x: np.ndarray, k) -> np.ndarray:
    k = int(k)
    if k not in _nc_cache:
        _nc_cache[k] = _build(k)
    nc = _nc_cache[k]

    x = np.ascontiguousarray(x, dtype=np.float32)
    in_maps = [
        {"x": x[i * ROWS_PER_CORE : (i + 1) * ROWS_PER_CORE]} for i in range(N_CORES)
    ]
    res = run_bass_kernel_spmd(nc, in_maps, core_ids=list(range(N_CORES)))
    return np.concatenate([r["out"] for r in res.results], axis=0)
